# revision 6
# baseline (speedup 1.0000x reference)
"""Trainium2 kernel for the InterpretedFlockingModel GNN message-passing problem.

Strategy
--------
The per-edge message is *linear* in (pos_dst, pos_src), so the edge phase
collapses to one value-dependent segmented reduction per node:
    S_i = sum_{e: dst(e)=i, src!=dst} pos[src(e)]            (2 channels)
plus two pure index statistics (in-degree incl./excl. self-loops) that the
host computes from the edge list while sharding.

Host-side sharding/layout prep (numpy, index work + layout only):
  * nodes are dealt round-robin by degree rank across the 8 cores and
    grouped into degree classes (pad to 64 / 96 / 128 slots), shrinking the
    padded payload ~40% vs uniform 128-slot padding,
  * each non-self edge's pos[src] fp16 payload lands at (dst-node, slot),
  * self-loop edges are dropped (the reference zeroes their messages).

Device kernel (per core, SPMD over 8 NeuronCores, no collectives since each
core owns all edges of its node range):
  * stream the padded payload (~4MB/core, the memory-bound part),
  * TensorE does the segmented reduction fused with the B-coefficient mix:
    a [k,128] fp16 stationary block holds x-slots (first half of k) and
    y-slots (second half) of 128 nodes; rhs [k,3] holds the matching B rows,
    so PSUM accumulates B @ S per node directly (k-blocks of one node
    accumulate via start/stop),
  * VectorE computes the per-node A-term, mean normalization and the
    elementwise update (all model FLOPs stay on device),
  * output [2, 128, G] per core, reassembled (unpermute/de-pad) on host.

Node position l on a core -> partition p = l % 128, group g = l // 128;
PSUM column 3*g + i holds msg-channel i of node (p, g).
"""

import numpy as np

N = 100000
E = 6400000
NCORES = 8
NPC = N // NCORES          # nodes per core
P = 128

_CACHE = {}


# ----------------------------------------------------------------- constants
def _msg_rows(x0, x1, x4, x5):
    # channels (m0, m1, m3) of the reference message fn; m2 feeds y4 which is
    # never consumed by the update, so it is dropped.
    d = x4 - x0
    m0 = (d + (x1 - x5) * 0.40914905) * 0.028998906
    m1 = (d + (x1 - x5) * 0.5819344) * -0.02637788
    m3 = (x1 * 0.95594215 - x5 - x0 * 0.20244296 - x4 * -0.17809269) * 0.026933579
    return np.array([m0, m1, m3], np.float64)


def _mats():
    A3 = np.stack([_msg_rows(1, 0, 0, 0), _msg_rows(0, 1, 0, 0)], 1)  # [3,2]
    B3 = np.stack([_msg_rows(0, 0, 1, 0), _msg_rows(0, 0, 0, 1)], 1)  # [3,2]
    return A3, B3


def _constants():
    A3, _ = _mats()
    # final preds are affine in basis [px,py,vx,vy, vy2,y6,y7,w], w = y7^2*y5
    U = np.zeros((4, 8), np.float64)  # u0..u3 over basis
    U[0, 1] = -0.0020586958                    # py
    U[0, 6] = -0.0020586958 / 0.037233025      # y7
    U[1, 0] = -0.10450508 * 0.015168043        # px
    U[1, 7] = +0.10450508 * 0.015168043        # w
    U[1, 5] = +0.10450508                      # y6
    U[2, 1] = -0.075265266 * 0.027931638       # py
    U[2, 5] = +0.075265266                     # y6
    U[2, 6] = +0.075265266                     # y7
    U[3, 2] = -0.08554904                      # vx
    U[3, 3] = +0.08554904                      # vy
    U[3, 4] = -0.08554904                      # vy2
    U[3, 5] = +0.08554904                      # y6
    U[3, 6] = +0.08554904 * 0.33928046         # y7
    PC = np.array(
        [
            [2 * -0.24326763, -1.0 / 0.7301285, -1.1234615, -0.24326763],
            [-1.0, -1.0, 1.0, 1.0],
        ],
        np.float64,
    )
    K = PC @ U  # [2 preds, 8 basis]

    consts = np.zeros((P, 28), np.float32)
    consts[:, 0:6] = A3.reshape(-1).astype(np.float32)   # CA  [i,j] row-major
    consts[:, 12:20] = K[:, 0:4].reshape(-1).astype(np.float32)  # basis px,py,vx,vy
    consts[:, 20:28] = K[:, 4:8].reshape(-1).astype(np.float32)  # basis vy2,y6,y7,w
    return consts


def _bmats():
    _, B3 = _mats()
    bm128 = np.zeros((P, 3), np.float16)   # split @ 64: x rows then y rows
    bm128[:64, :] = B3[:, 0].astype(np.float16)
    bm128[64:, :] = B3[:, 1].astype(np.float16)
    bm64 = np.zeros((64, 3), np.float16)   # split @ 32
    bm64[:32, :] = B3[:, 0].astype(np.float16)
    bm64[32:, :] = B3[:, 1].astype(np.float16)
    return bm128, bm64


# ----------------------------------------------------------------- host prep
def _prep(pos, vel, edge_index):
    pos = np.ascontiguousarray(np.asarray(pos, np.float32))
    vel = np.ascontiguousarray(np.asarray(vel, np.float32))
    ei = np.asarray(edge_index)
    src = ei[0].astype(np.int64)
    dst = ei[1].astype(np.int64)

    cnt = np.bincount(dst, minlength=N).astype(np.float32)  # incl self-loops
    keep = src != dst
    src2 = src[keep]
    dst2 = dst[keep]
    nns_a = np.bincount(dst2, minlength=N).astype(np.float32)
    inv_a = (1.0 / np.maximum(cnt, 1.0)).astype(np.float32)
    deg2 = np.bincount(dst2, minlength=N)
    assert deg2.max() <= 128, f"max non-self degree {deg2.max()} exceeds 128"

    # deal nodes round-robin by degree rank -> equal class mix per core
    rank = np.argsort(deg2, kind="stable")[::-1]          # degree descending
    core_of = np.empty(N, np.int64)
    core_of[rank] = np.arange(N) % NCORES
    cls_of = np.where(deg2 <= 64, 0, np.where(deg2 <= 96, 1, 2))  # 0:64 1:96 2:128

    # per-core class counts -> common padded group counts
    counts = np.zeros((NCORES, 3), np.int64)
    for c in range(3):
        counts[:, c] = np.bincount(core_of[cls_of == c], minlength=NCORES)
    gcls = [int(np.ceil(counts[:, c].max() / P)) for c in range(3)]
    G64, G96, G128 = (max(g, 1) for g in gcls)
    GT = G64 + G96 + G128
    PADN = P * GT

    # position of each node inside its core: class64 block, then 96, then 128
    ofs = np.array([0, P * G64, P * (G64 + G96)], np.int64)
    pos_l = np.empty(N, np.int64)
    nodelists = np.full((NCORES, PADN), -1, np.int64)
    for c in range(NCORES):
        mine = np.flatnonzero(core_of == c)
        for k in range(3):
            sel = mine[cls_of[mine] == k]
            pos_l[sel] = ofs[k] + np.arange(sel.size)
            nodelists[c, ofs[k]:ofs[k] + sel.size] = sel

    # --- edge payload scatter into class regions (fp16) ---
    order = np.argsort(dst2, kind="stable")
    ds = dst2[order]
    ss = src2[order]
    starts = np.zeros(N, np.int64)
    np.cumsum(deg2[:-1], out=starts[1:])
    slot = np.arange(ds.size, dtype=np.int64) - starts[ds]
    ecore = core_of[ds]
    el = pos_l[ds]
    ecls = cls_of[ds]
    vx_ = pos[ss, 0].astype(np.float16)
    vy_ = pos[ss, 1].astype(np.float16)

    n64, n96, n128 = P * G64, P * G96, P * G128
    V64 = np.zeros((NCORES, P, n64), np.float16)
    V96a = np.zeros((NCORES, P, n96), np.float16)
    V96b = np.zeros((NCORES, 64, n96), np.float16)
    V128a = np.zeros((NCORES, P, n128), np.float16)
    V128b = np.zeros((NCORES, P, n128), np.float16)

    m = ecls == 0
    V64[ecore[m], slot[m], el[m]] = vx_[m]
    V64[ecore[m], 64 + slot[m], el[m]] = vy_[m]
    for clsk, Va, Vb, off, ksplit in ((1, V96a, V96b, P * G64, 32),
                                      (2, V128a, V128b, P * (G64 + G96), 64)):
        m = ecls == clsk
        lo = m & (slot < 64)
        hi = m & (slot >= 64)
        Va[ecore[lo], slot[lo], el[lo] - off] = vx_[lo]
        Va[ecore[lo], 64 + slot[lo], el[lo] - off] = vy_[lo]
        Vb[ecore[hi], slot[hi] - 64, el[hi] - off] = vx_[hi]
        Vb[ecore[hi], ksplit + slot[hi] - 64, el[hi] - off] = vy_[hi]

    # --- aux planes in permuted node order ---
    def plane(arr):
        a = np.asarray(arr, np.float32)
        out = np.zeros((NCORES, PADN), np.float32)
        valid = nodelists >= 0
        out[valid] = a[nodelists[valid]]
        return out.reshape(NCORES, GT, P).transpose(0, 2, 1)

    X = np.stack([plane(pos[:, 0]), plane(pos[:, 1]), plane(vel[:, 0]),
                  plane(vel[:, 1]), plane(nns_a), plane(inv_a)], axis=1)
    X = np.ascontiguousarray(X)  # [NCORES, 6, P, GT]

    meta = dict(G64=G64, G96=G96, G128=G128, GT=GT, PADN=PADN,
                nodelists=nodelists)
    return (V64.reshape(NCORES, P, n64), V96a.reshape(NCORES, P, n96),
            V96b.reshape(NCORES, 64, n96), V128a.reshape(NCORES, P, n128),
            V128b.reshape(NCORES, P, n128), X, meta)


# ------------------------------------------------------------- device kernel
def _chunk_plan(G64, G96, G128):
    """Static chunk schedule: (class, group_offset_in_class, ngroups)."""
    plan = []

    def chunks(cls, total, sizes_first, size_rest):
        done = 0
        for s in sizes_first:
            if done >= total:
                return
            s = min(s, total - done)
            plan.append((cls, done, s))
            done += s
        while done < total:
            s = min(size_rest, total - done)
            plan.append((cls, done, s))
            done += s

    chunks(0, G64, [4, 12], 22)     # 32KB/group: small first chunks for fast start
    chunks(1, G96, [], 15)          # 48KB/group
    chunks(2, G128, [], 10)         # 64KB/group
    return plan


def _build_nc(G64, G96, G128):
    import concourse.bacc as bacc
    import concourse.tile as tile
    from concourse import mybir

    f32 = mybir.dt.float32
    f16 = mybir.dt.float16
    ADD = mybir.AluOpType.add
    AX = mybir.AxisListType.X

    GT = G64 + G96 + G128
    n64, n96, n128 = P * G64, P * G96, P * G128

    nc = bacc.Bacc("TRN2", target_bir_lowering=False, debug=False,
                   enable_asserts=False, num_devices=NCORES)
    V64d = nc.dram_tensor("v64", [P, n64], f16, kind="ExternalInput")
    V96ad = nc.dram_tensor("v96a", [P, n96], f16, kind="ExternalInput")
    V96bd = nc.dram_tensor("v96b", [64, n96], f16, kind="ExternalInput")
    V128ad = nc.dram_tensor("v128a", [P, n128], f16, kind="ExternalInput")
    V128bd = nc.dram_tensor("v128b", [P, n128], f16, kind="ExternalInput")
    Xd = nc.dram_tensor("x", [6, P, GT], f32, kind="ExternalInput")
    Cd = nc.dram_tensor("c", [P, 28], f32, kind="ExternalInput")
    B128d = nc.dram_tensor("bm128", [P, 3], f16, kind="ExternalInput")
    B64d = nc.dram_tensor("bm64", [64, 3], f16, kind="ExternalInput")
    Od = nc.dram_tensor("o", [2, P, GT], f32, kind="ExternalOutput")

    plan = _chunk_plan(G64, G96, G128)
    cls_gofs = [0, G64, G64 + G96]      # class -> global group offset
    # psum half split at the chunk boundary nearest GT/2
    acc, G0 = 0, 0
    for cls, gofs, ng in plan:
        if acc + ng > GT // 2 and G0 == 0:
            G0 = acc
        acc += ng
    if G0 == 0:
        G0 = acc // 2
    G1 = GT - G0

    with tile.TileContext(nc) as tc:
        with tc.tile_pool(name="vp", bufs=3) as vp, \
             tc.tile_pool(name="mp", bufs=1) as mp, \
             tc.tile_pool(name="pp", bufs=1, space="PSUM") as pp:
            # small side inputs ride the Activation HWDGE ring so the V
            # chunks get the Sync ring to themselves from t=0
            bmt = mp.tile([P, 3], f16, tag="bmt")
            nc.scalar.dma_start(out=bmt[:], in_=B128d[:])
            bmt64 = mp.tile([64, 3], f16, tag="bmt64")
            nc.scalar.dma_start(out=bmt64[:], in_=B64d[:])
            aux = mp.tile([P, 6 * GT], f32, tag="aux")
            nc.scalar.dma_start(
                out=aux[:].rearrange("p (c g) -> p c g", c=6),
                in_=Xd[:].rearrange("c p g -> p c g"),
            )
            ct = mp.tile([P, 28], f32, tag="ct")
            nc.scalar.dma_start(out=ct[:], in_=Cd[:])

            # --- edge phase: PSUM[:, 3g+i] = (B @ S)_i for node (p, g) ---
            ps0 = pp.tile([P, 3 * G0], f32, tag="ps0")
            ps1 = pp.tile([P, 3 * G1], f32, tag="ps1")

            def psum_ap(g):
                return (ps0, g) if g < G0 else (ps1, g - G0)

            for cls, cofs, ng in plan:
                w = ng * P
                if cls == 0:
                    vt = vp.tile([P, w], f16, tag=f"vt64_{ng}")
                    nc.sync.dma_start(out=vt[:], in_=V64d[:, cofs * P:cofs * P + w])
                    blocks = [(vt, bmt, P)]
                elif cls == 1:
                    vt = vp.tile([P, w], f16, tag=f"vt96a_{ng}")
                    nc.sync.dma_start(out=vt[:], in_=V96ad[:, cofs * P:cofs * P + w])
                    vtb = vp.tile([64, w], f16, tag=f"vt96b_{ng}")
                    nc.sync.dma_start(out=vtb[:], in_=V96bd[:, cofs * P:cofs * P + w])
                    blocks = [(vt, bmt, P), (vtb, bmt64, 64)]
                else:
                    vt = vp.tile([P, w], f16, tag=f"vt128a_{ng}")
                    nc.sync.dma_start(out=vt[:], in_=V128ad[:, cofs * P:cofs * P + w])
                    vtb = vp.tile([P, w], f16, tag=f"vt128b_{ng}")
                    nc.sync.dma_start(out=vtb[:], in_=V128bd[:, cofs * P:cofs * P + w])
                    blocks = [(vt, bmt, P), (vtb, bmt, P)]
                nb = len(blocks)
                for gl in range(ng):
                    g = cls_gofs[cls] + cofs + gl
                    ps, go = psum_ap(g)
                    for bi, (tile_, bm_, kdim) in enumerate(blocks):
                        nc.tensor.matmul(
                            out=ps[:, 3 * go:3 * go + 3],
                            lhsT=tile_[:kdim, gl * P:(gl + 1) * P],
                            rhs=bm_[:kdim, 0:3],
                            start=(bi == 0),
                            stop=(bi == nb - 1),
                        )

            # --- node phase ---
            G = GT
            # qxy = [px|py] * nns
            q = mp.tile([P, 2 * G], f32, tag="q")
            nns_b = aux[:, 4 * G:5 * G][:, None, :].to_broadcast([P, 2, G])
            nc.vector.tensor_mul(
                out=q[:].rearrange("p (j g) -> p j g", j=2),
                in0=aux[:, 0:2 * G].rearrange("p (j g) -> p j g", j=2),
                in1=nns_b,
            )
            # sA[g,i] = sum_j CA[i,j] * q[j,g]
            mA = mp.tile([P, G * 3 * 2], f32, tag="mA")
            q_gij = q[:].rearrange("p (j g) -> p g j", j=2)[:, :, None, :] \
                .to_broadcast([P, G, 3, 2])
            CA_gij = ct[:, 0:6].rearrange("p (i j) -> p i j", i=3)[:, None, :, :] \
                .to_broadcast([P, G, 3, 2])
            nc.vector.tensor_mul(
                out=mA[:].rearrange("p (g i j) -> p g i j", i=3, j=2),
                in0=q_gij, in1=CA_gij,
            )
            sA = mp.tile([P, G * 3], f32, tag="sA")
            nc.vector.tensor_reduce(
                out=sA[:].rearrange("p (g i) -> p g i", i=3),
                in_=mA[:].rearrange("p (g i j) -> p g i j", i=3, j=2),
                axis=AX, op=ADD,
            )

            # preds = K0 @ [px,py,vx,vy] + K1 @ [vy2,y6,y7,w]
            def combo(src_ap, coef_ap, mtag, gh):
                m = mp.tile([P, 2 * gh * 4], f32, tag=mtag)
                src_qgb = src_ap.rearrange("p (b g) -> p g b", b=4)[:, None, :, :] \
                    .to_broadcast([P, 2, gh, 4])
                coef_qgb = coef_ap.rearrange("p (q b) -> p q b", q=2)[:, :, None, :] \
                    .to_broadcast([P, 2, gh, 4])
                nc.vector.tensor_mul(
                    out=m[:].rearrange("p (q g b) -> p q g b", q=2, b=4),
                    in0=src_qgb, in1=coef_qgb,
                )
                r = mp.tile([P, 2 * gh], f32, tag=mtag + "r")
                nc.vector.tensor_reduce(
                    out=r[:].rearrange("p (q g) -> p q g", q=2),
                    in_=m[:].rearrange("p (q g b) -> p q g b", q=2, b=4),
                    axis=AX, op=ADD,
                )
                return r

            # pA over the full width depends only on aux -> runs early
            pA = combo(aux[:, 0:4 * G], ct[:, 12:20], "mP0", G)
            pA_qg = pA[:].rearrange("p (q g) -> p q g", q=2)

            # psum-dependent tail, per half so half 0 overlaps the PE phase
            for gofs, gh, ps in ((0, G0, ps0), (G0, G1, ps1)):
                hx = f"h{gofs}"
                s = mp.tile([P, 3 * gh], f32, tag="s" + hx)
                nc.vector.tensor_add(out=s[:], in0=sA[:, 3 * gofs:3 * (gofs + gh)],
                                     in1=ps[:, 0:3 * gh])
                s_ig = s[:].rearrange("p (g i) -> p i g", i=3)

                bs = mp.tile([P, 4 * gh], f32, tag="bs" + hx)
                inv_b = aux[:, 5 * G + gofs:5 * G + gofs + gh][:, None, :] \
                    .to_broadcast([P, 2, gh])
                nc.vector.tensor_mul(       # y6,y7 = inv * (s0,s1)
                    out=bs[:, gh:3 * gh].rearrange("p (i g) -> p i g", i=2),
                    in0=s_ig[:, 0:2, :],
                    in1=inv_b,
                )
                vy = aux[:, 3 * G + gofs:3 * G + gofs + gh]
                nc.vector.tensor_mul(out=bs[:, 0:gh], in0=vy, in1=vy)      # vy2
                t7 = mp.tile([P, gh], f32, tag="t7" + hx)
                y7 = bs[:, 2 * gh:3 * gh]
                nc.vector.tensor_mul(out=t7[:], in0=y7, in1=y7)            # y7^2
                nc.vector.tensor_mul(out=bs[:, 3 * gh:4 * gh], in0=t7[:],
                                     in1=s_ig[:, 2, :])                    # w = y7^2*y5

                pB = combo(bs[:], ct[:, 20:28], "mP1" + hx, gh)
                ot = mp.tile([P, 2 * gh], f32, tag="ot" + hx)
                nc.vector.tensor_add(
                    out=ot[:].rearrange("p (c g) -> p c g", c=2),
                    in0=pA_qg[:, :, gofs:gofs + gh],
                    in1=pB[:].rearrange("p (c g) -> p c g", c=2),
                )
                nc.sync.dma_start(
                    out=Od[:].rearrange("c p g -> p c g")[:, :, gofs:gofs + gh],
                    in_=ot[:].rearrange("p (c g) -> p c g", c=2),
                )
    nc.finalize()
    return nc


# -------------------------------------------------------------------- driver
def _run(pos, vel, edge_index, trace=False, trace_kwargs=None):
    from concourse.bass_utils import run_bass_kernel_spmd

    V64, V96a, V96b, V128a, V128b, X, meta = _prep(pos, vel, edge_index)
    key = (meta["G64"], meta["G96"], meta["G128"])
    if _CACHE.get("key") != key:
        _CACHE["nc"] = _build_nc(*key)
        _CACHE["key"] = key
    nc = _CACHE["nc"]

    consts = _constants()
    bm128, bm64 = _bmats()
    in_maps = [
        {"v64": V64[i], "v96a": V96a[i], "v96b": V96b[i],
         "v128a": V128a[i], "v128b": V128b[i],
         "x": X[i], "c": consts, "bm128": bm128, "bm64": bm64}
        for i in range(NCORES)
    ]
    res = run_bass_kernel_spmd(
        nc, in_maps, list(range(NCORES)), trace=trace,
        **({"trace_kwargs": trace_kwargs} if trace_kwargs else {}),
    )

    GT, PADN = meta["GT"], meta["PADN"]
    nodelists = meta["nodelists"]
    out = np.empty((N, 2), np.float32)
    for i in range(NCORES):
        o = np.asarray(res.results[i]["o"])  # [2, P, GT]
        flat = o.transpose(0, 2, 1).reshape(2, PADN)  # index by l = g*128+p
        valid = nodelists[i] >= 0
        out[nodelists[i][valid], 0] = flat[0, valid]
        out[nodelists[i][valid], 1] = flat[1, valid]
    return out, res


def kernel(pos, vel, edge_index):
    out, _ = _run(pos, vel, edge_index, trace=False)
    return out
